# revision 2
# baseline (speedup 1.0000x reference)
"""CPC loss (nn_CPCLossV2) Trainium2 Bass kernel.

v1 staged host-pregathered negative rows (16.8 MB/core -> 137 MB of H2D per
call, which dominates at the ~60 MB/s axon tunnel rate).  v2 ships only each
core's own bf16 embedding shard (+ W shard + negative indices, ~1.1 MB/core,
9 MB total), reconstructs the full table on device with an HBM AllGather over
NeuronLink, and does the negative-row gather ON DEVICE with the GPSIMD
ap_gather library op (InstAPGather; the column-gather layout keeps one shared
index list per 16-partition group, which matches its wrapped-index contract).

Layouts:
  table [128, 16384, 2] bf16  table[p, r, d] = emb[r, 128*d + p]
    (transposed + h-half interleaved so one 4-byte ap_gather unit carries
     both halves of the h dimension for a given row)
  own   [128, 2048, 2]  bf16  same layout, core's own rows (from the local
     input, since SPMD APs cannot depend on the core id)
  predsTI [128, 512, 2] bf16  predsTI[p, g, d] = predicts[g, 128*d + p]
  slot s of the gather: s = (B*64 + j)*128 + p  ->  group 128*B + p, neg j.
    A 128-slot tile therefore multiplies ELEMENTWISE with the band's predsTI
    tile, and two accumulating PE transposes + one free-axis reduce produce
    nlt[g(part), B*64+j] — the same logit layout the v1 logsumexp tail used.

Per-group loss: logsumexp([pos, neg_0..63]) - pos; partial sums [128, 1] per
core are combined on host (sum / 4096).
"""

import os
from contextlib import ExitStack

import numpy as np
import ml_dtypes

N = 4096          # groups
K = 4             # rows per group
H = 256           # embedding dim
M = 64            # negatives per group
NCORES = 8
S = N // NCORES   # 512 groups per core
ROWS = S * K      # 2048 local rows
ALLROWS = N * K   # 16384
BANDS = S // 128  # 4 bands of 128 groups
SLOTS = S * M     # 32768 gathered rows per core
NCHUNK = 8        # gather chunks per core
CH_SLOTS = SLOTS // NCHUNK          # 4096 slots per chunk
CH_TILES = CH_SLOTS // 128          # 32 slot-tiles per chunk
WSH = (K - 1) * H // NCORES         # 96 W rows per core

# single packed input blob per core (bytes, 4-aligned sections)
OFF_EMB = 0
SZ_EMB = ROWS * H                    # int8
OFF_SCL = OFF_EMB + SZ_EMB
SZ_SCL = ROWS * 4                    # f32
OFF_W = OFF_SCL + SZ_SCL
SZ_W = WSH * H * 2                   # bf16
OFF_IDX = OFF_W + SZ_W
SZ_IDX = SLOTS * 2                   # i16
OFF_B = OFF_IDX + SZ_IDX
SZ_B = H * 4                         # f32
BLOB = OFF_B + SZ_B                  # 648192
SZ_ES = OFF_SCL + SZ_SCL             # emb+scl section, all-gathered together

_CACHE = {}


# --------------------------------------------------------------------------
# device program
# --------------------------------------------------------------------------

def build_nc(debug=False):
    import concourse.bass as bass
    import concourse.tile as tile
    from concourse import bacc, masks, mybir

    f32 = mybir.dt.float32
    bf16 = mybir.dt.bfloat16
    i16 = mybir.dt.int16
    Alu = mybir.AluOpType
    Act = mybir.ActivationFunctionType
    Ax = mybir.AxisListType

    nc = bacc.Bacc(
        "TRN2", target_bir_lowering=False, debug=debug, num_devices=NCORES
    )

    i8 = mybir.dt.int8
    u8 = mybir.dt.uint8
    blob = nc.dram_tensor("blob", [BLOB], u8, kind="ExternalInput").ap()
    emb_in = blob[OFF_EMB : OFF_EMB + SZ_EMB].bitcast(i8).rearrange(
        "(r c) -> r c", c=H
    )
    scl_in = blob[OFF_SCL : OFF_SCL + SZ_SCL].bitcast(f32).rearrange(
        "(t p) -> p t", p=128
    )
    w_in = blob[OFF_W : OFF_W + SZ_W].bitcast(bf16).rearrange("(r c) -> r c", c=H)
    idx_in = blob[OFF_IDX : OFF_IDX + SZ_IDX].bitcast(i16).rearrange(
        "(a b) -> a b", b=SLOTS // 16
    )
    bvec = blob[OFF_B : OFF_B + SZ_B].bitcast(f32).rearrange("(h o) -> h o", o=1)
    lossp = nc.dram_tensor("loss_part", [128, 1], f32, kind="ExternalOutput").ap()

    cc_es = nc.dram_tensor("cc_es", [SZ_ES], u8, kind="Internal").ap()
    cc_w = nc.dram_tensor("cc_w", [WSH, H], bf16, kind="Internal").ap()
    ag_es = nc.dram_tensor(
        "ag_es", [NCORES * SZ_ES], u8, kind="Internal", addr_space="Shared"
    ).ap()
    ag_w = nc.dram_tensor(
        "ag_w", [(K - 1) * H, H], bf16, kind="Internal", addr_space="Shared"
    ).ap()

    with tile.TileContext(nc) as tc, ExitStack() as ctx:
        cpool = ctx.enter_context(tc.tile_pool(name="const", bufs=1))
        npool = ctx.enter_context(tc.tile_pool(name="nat", bufs=4))
        gpool = ctx.enter_context(tc.tile_pool(name="gather", bufs=3))
        ppool = ctx.enter_context(tc.tile_pool(name="prod", bufs=4))
        pspool = ctx.enter_context(tc.tile_pool(name="psA", bufs=2, space="PSUM"))
        tpool = ctx.enter_context(tc.tile_pool(name="psB", bufs=2, space="PSUM"))
        t16pool = ctx.enter_context(tc.tile_pool(name="psC", bufs=2, space="PSUM"))

        # ---- stage inputs to Internal DRAM and AllGather ---------------------
        stage_e = cpool.tile([128, SZ_ES // 128], u8, tag="stage_e")
        nc.sync.dma_start(
            out=stage_e[:],
            in_=blob[0:SZ_ES].rearrange("(p x) -> p x", p=128),
        )
        nc.sync.dma_start(
            out=cc_es.rearrange("(p x) -> p x", p=128), in_=stage_e[:]
        )
        stage_w = cpool.tile([WSH, H], bf16, tag="stage_w")
        nc.sync.dma_start(out=stage_w[:], in_=w_in)
        nc.sync.dma_start(out=cc_w, in_=stage_w[:])
        scl_own = cpool.tile([128, ROWS // 128], f32, tag="scl_own")
        nc.sync.dma_start(out=scl_own[:], in_=scl_in)
        grp = [list(range(NCORES))]
        nc.gpsimd.collective_compute(
            "AllGather", Alu.bypass, replica_groups=grp, ins=[cc_es], outs=[ag_es]
        )
        nc.gpsimd.collective_compute(
            "AllGather", Alu.bypass, replica_groups=grp, ins=[cc_w], outs=[ag_w]
        )

        ident = cpool.tile([128, 128], f32, tag="ident")
        masks.make_identity(nc, ident[:])
        ident16 = cpool.tile([128, 128], bf16, tag="ident16")
        nc.vector.tensor_copy(ident16[:], ident[:])

        bias_sb = []
        for hc in range(2):
            t = cpool.tile([128, 1], f32, tag=f"bias{hc}")
            nc.sync.dma_start(out=t[:], in_=bvec[128 * hc : 128 * (hc + 1), :])
            bias_sb.append(t)

        # ---- own table [128, 2048, 2] from the local shard ------------------
        own = cpool.tile([128, ROWS, 2], bf16, tag="own")
        for rt in range(ROWS // 128):
            ntq = npool.tile([128, H], i8)
            nc.sync.dma_start(out=ntq[:], in_=emb_in[128 * rt : 128 * (rt + 1), :])
            nt = npool.tile([128, H], bf16)
            nc.scalar.activation(
                nt[:], ntq[:], Act.Copy, scale=scl_own[:, rt : rt + 1]
            )
            for hc in range(2):
                ps = t16pool.tile([128, 128], bf16, tag="tps16")
                nc.tensor.transpose(ps[:], nt[:, 128 * hc : 128 * (hc + 1)], ident16[:])
                nc.vector.tensor_copy(own[:, 128 * rt : 128 * (rt + 1), hc], ps[:])

        # ---- W chunks from the AllGather ------------------------------------
        W_sb = []
        for kc in range(6):
            t = cpool.tile([128, H], bf16, tag=f"W{kc}")
            nc.sync.dma_start(out=t[:], in_=ag_w[128 * kc : 128 * (kc + 1), :])
            W_sb.append(t)

        # ---- predsTI = (hist_x @ W + b)^T, interleaved -----------------------
        # hist_x^T[j*256+h, g] = emb[4g+j, h] = own[h%128, 4g+j, h//128]
        ownr = own[:].rearrange("p (g j) d -> p j g d", j=K)
        predsTI = cpool.tile([128, S, 2], bf16, tag="predsTI")
        for mc in range(2):
            pt = pspool.tile([128, S], f32, tag="predsT_ps")
            for j in range(K - 1):
                for hc in range(2):
                    kc = 2 * j + hc
                    nc.tensor.matmul(
                        pt[:],
                        lhsT=W_sb[kc][:, 128 * mc : 128 * (mc + 1)],
                        rhs=ownr[:, j, :, hc],
                        start=(kc == 0),
                        stop=(kc == 5),
                    )
            nc.vector.tensor_scalar_add(predsTI[:, :, mc], pt[:], bias_sb[mc][:])

        # ---- positive logits -------------------------------------------------
        # histyT[p, g, d] = own[p, 4g+3, d]
        pos_t = cpool.tile([128, BANDS], f32, tag="pos_t")
        pprod = cpool.tile([128, S, 2], f32, tag="pprod")
        nc.vector.tensor_tensor(pprod[:], predsTI[:], ownr[:, K - 1, :, :], op=Alu.mult)
        for B in range(BANDS):
            ps = tpool.tile([128, 128], f32, tag="tps")
            for d in range(2):
                nc.tensor.matmul(
                    ps[:],
                    lhsT=pprod[:, 128 * B : 128 * (B + 1), d],
                    rhs=ident[:],
                    is_transpose=True,
                    start=(d == 0),
                    stop=(d == 1),
                )
            nc.vector.tensor_reduce(pos_t[:, B : B + 1], ps[:], axis=Ax.X, op=Alu.add)

        # ---- full table [128, 16384, 2] from the AllGather -------------------
        scl_all = cpool.tile([128, ALLROWS // 128], f32, tag="scl_all")
        TPC = ROWS // 128  # 16 row-tiles per core section
        for ci in range(NCORES):
            sec = ci * SZ_ES
            nc.sync.dma_start(
                out=scl_all[:, TPC * ci : TPC * (ci + 1)],
                in_=ag_es[sec + OFF_SCL : sec + OFF_SCL + SZ_SCL]
                .bitcast(f32)
                .rearrange("(t p) -> p t", p=128),
            )
        table = cpool.tile([128, ALLROWS, 2], bf16, tag="table")
        for rt in range(ALLROWS // 128):
            ci, lt = rt // TPC, rt % TPC
            sec = ci * SZ_ES
            ntq = npool.tile([128, H], i8)
            nc.sync.dma_start(
                out=ntq[:],
                in_=ag_es[
                    sec + 128 * lt * H : sec + 128 * (lt + 1) * H
                ]
                .bitcast(i8)
                .rearrange("(r c) -> r c", c=H),
            )
            nt = npool.tile([128, H], bf16)
            nc.scalar.activation(
                nt[:], ntq[:], Act.Copy, scale=scl_all[:, rt : rt + 1]
            )
            for hc in range(2):
                ps = t16pool.tile([128, 128], bf16, tag="tps16")
                nc.tensor.transpose(ps[:], nt[:, 128 * hc : 128 * (hc + 1)], ident16[:])
                nc.vector.tensor_copy(table[:, 128 * rt : 128 * (rt + 1), hc], ps[:])

        # ---- negative indices (replicated into all 8 16-partition groups) ----
        idx_sb = cpool.tile([128, SLOTS // 16], i16, tag="idx")
        for g8 in range(8):
            nc.sync.dma_start(out=idx_sb[16 * g8 : 16 * (g8 + 1), :], in_=idx_in)

        # ---- gather + negative logits ---------------------------------------
        nlt = cpool.tile([128, BANDS * M], f32, tag="nlt")
        QW = CH_SLOTS // 16  # idx words per chunk per partition
        for ci in range(NCHUNK):
            B = ci // 2
            G = gpool.tile([128, CH_SLOTS, 2], bf16)
            nc.gpsimd.ap_gather(
                G[:],
                table[:],
                idx_sb[:, QW * ci : QW * (ci + 1)],
                channels=128,
                num_elems=ALLROWS,
                d=2,
                num_idxs=CH_SLOTS,
            )
            for t in range(CH_TILES):
                c = ci * CH_TILES + t  # global tile = B*64 + j
                P = ppool.tile([128, 128, 2], f32)
                nc.vector.tensor_tensor(
                    P[:],
                    G[:, 128 * t : 128 * (t + 1), :],
                    predsTI[:, 128 * B : 128 * (B + 1), :],
                    op=Alu.mult,
                )
                ps = tpool.tile([128, 128], f32, tag="tps")
                for d in range(2):
                    nc.tensor.matmul(
                        ps[:],
                        lhsT=P[:, :, d],
                        rhs=ident[:],
                        is_transpose=True,
                        start=(d == 0),
                        stop=(d == 1),
                    )
                scr = ppool.tile([128, 128], f32)
                nc.scalar.activation(
                    scr[:], ps[:], Act.Copy, accum_out=nlt[:, c : c + 1]
                )

        # ---- per-group logsumexp and loss (as v1) ---------------------------
        fpool = ctx.enter_context(tc.tile_pool(name="fin", bufs=1))
        mx = fpool.tile([128, BANDS], f32, tag="mx")
        nc.vector.tensor_reduce(
            mx[:], nlt[:].rearrange("p (b j) -> p b j", b=BANDS),
            axis=Ax.X, op=Alu.max,
        )
        nc.vector.tensor_tensor(mx[:], mx[:], pos_t[:], op=Alu.max)
        negmx = fpool.tile([128, BANDS], f32, tag="negmx")
        nc.vector.tensor_scalar_mul(negmx[:], mx[:], -1.0)
        sume = fpool.tile([128, BANDS], f32, tag="sume")
        scr2 = fpool.tile([128, M], f32, tag="scr2")
        for B in range(BANDS):
            nc.scalar.activation(
                scr2[:],
                nlt[:, M * B : M * (B + 1)],
                Act.Exp,
                bias=negmx[:, B : B + 1],
                accum_out=sume[:, B : B + 1],
            )
        pd = fpool.tile([128, BANDS], f32, tag="pd")
        nc.vector.tensor_tensor(pd[:], pos_t[:], mx[:], op=Alu.subtract)
        pexp = fpool.tile([128, BANDS], f32, tag="pexp")
        nc.scalar.activation(pexp[:], pd[:], Act.Exp)
        tot = fpool.tile([128, BANDS], f32, tag="tot")
        nc.vector.tensor_tensor(tot[:], sume[:], pexp[:], op=Alu.add)
        lse = fpool.tile([128, BANDS], f32, tag="lse")
        nc.scalar.activation(lse[:], tot[:], Act.Ln)
        # loss_pg = lse + mx - pos
        nc.vector.tensor_tensor(lse[:], lse[:], mx[:], op=Alu.add)
        nc.vector.tensor_tensor(lse[:], lse[:], pos_t[:], op=Alu.subtract)
        lred = fpool.tile([128, 1], f32, tag="lred")
        nc.vector.tensor_reduce(lred[:], lse[:], axis=Ax.X, op=Alu.add)
        nc.sync.dma_start(out=lossp, in_=lred[:])

    nc.compile()
    return nc


# --------------------------------------------------------------------------
# host-side sharding
# --------------------------------------------------------------------------

def _neg_indices(target, perm, k, m):
    """neg_idx[g, j] = cand[g][perm[g, j]] exactly as the reference builds it."""
    n = target.shape[0] // k
    t64 = np.asarray(target)
    expected = np.repeat(np.arange(n, dtype=t64.dtype), k)
    p = np.asarray(perm)[:, :m].astype(np.int64)
    if np.array_equal(t64, expected):
        # cand[g][j] = j if j < k*g else j + k
        g = np.arange(n, dtype=np.int64)[:, None]
        return p + k * (p >= k * g)
    # generic (slow) fallback, matches jnp.where(..., size=k*(n-1), fill=0)
    group_t = t64[0::k]
    out = np.zeros((n, m), dtype=np.int64)
    order = np.arange(t64.shape[0], dtype=np.int64)
    for gi in range(n):
        cand = order[t64 != group_t[gi]]
        cand = np.pad(cand, (0, k * (n - 1) - cand.shape[0]))
        out[gi] = cand[p[gi]]
    return out


def _prep_inputs(embeddings, W, b, target, perm, k, m):
    embf = np.asarray(embeddings, dtype=np.float32)
    amax = np.abs(embf).max(axis=1, keepdims=True)
    scales = (amax / 127.0 + 1e-30).astype(np.float32)        # [N*K, 1]
    emb_q = np.rint(embf / scales).astype(np.int8)
    W16 = np.asarray(W, dtype=np.float32).astype(ml_dtypes.bfloat16)
    bf = np.asarray(b, dtype=np.float32).reshape(H, 1)
    neg_idx = _neg_indices(target, perm, k, m)  # [N, M] global row ids

    in_maps = []
    for c in range(NCORES):
        ni = neg_idx[S * c : S * (c + 1)]                      # [512, 64] (g, j)
        # slot s = (B*64 + j)*128 + p  ->  (band B, neg j, partition p)
        lin = (
            ni.reshape(BANDS, 128, M).transpose(0, 2, 1).reshape(SLOTS)
        )
        idxw = np.ascontiguousarray(
            lin.reshape(SLOTS // 16, 16).T.astype(np.int16)
        )                                                       # [16, 2048]
        parts = [
            np.ascontiguousarray(emb_q[ROWS * c : ROWS * (c + 1)]).view(np.uint8).ravel(),
            np.ascontiguousarray(scales[ROWS * c : ROWS * (c + 1)]).view(np.uint8).ravel(),
            np.ascontiguousarray(W16[WSH * c : WSH * (c + 1)]).view(np.uint8).ravel(),
            idxw.view(np.uint8).ravel(),
            np.ascontiguousarray(bf).view(np.uint8).ravel(),
        ]
        in_maps.append({"blob": np.concatenate(parts)})
    return in_maps


def _get_runner():
    """Compile once; return a callable(in_maps) -> list of per-core out dicts.

    Same execution path as run_bass_kernel_spmd under axon (shard_map over
    a _bass_exec_p custom call), but the traced/jitted executable is cached
    across calls instead of being rebuilt per invocation.
    """
    if "runner" in _CACHE:
        return _CACHE["runner"]

    import jax
    from jax.sharding import Mesh, PartitionSpec
    from jax.experimental.shard_map import shard_map
    from concourse import mybir
    from concourse.bass2jax import (
        _bass_exec_p, partition_id_tensor, install_neuronx_cc_hook,
    )

    if "nc" not in _CACHE:
        _CACHE["nc"] = build_nc(debug=False)
    nc = _CACHE["nc"]
    install_neuronx_cc_hook()

    partition_name = nc.partition_id_tensor.name if nc.partition_id_tensor else None
    in_names, out_names, out_avals = [], [], []
    for alloc in nc.m.functions[0].allocations:
        if not isinstance(alloc, mybir.MemoryLocationSet):
            continue
        name = alloc.memorylocations[0].name
        if alloc.kind == "ExternalInput":
            if name != partition_name:
                in_names.append(name)
        elif alloc.kind == "ExternalOutput":
            out_names.append(name)
            out_avals.append(
                jax.core.ShapedArray(
                    tuple(alloc.tensor_shape), mybir.dt.np(alloc.dtype)
                )
            )
    n_params, n_outs = len(in_names), len(out_avals)
    in_names_all = in_names + out_names + (
        [partition_name] if partition_name else []
    )
    donate = tuple(range(n_params, n_params + n_outs))

    def _body(*args):
        operands = list(args)
        if partition_name is not None:
            operands.append(partition_id_tensor())
        return tuple(
            _bass_exec_p.bind(
                *operands,
                out_avals=tuple(out_avals),
                in_names=tuple(in_names_all),
                out_names=tuple(out_names),
                lowering_input_output_aliases=(),
                sim_require_finite=True,
                sim_require_nnan=True,
                nc=nc,
            )
        )

    mesh = Mesh(np.asarray(jax.devices()[:NCORES]), ("core",))
    sharded = jax.jit(
        shard_map(
            _body,
            mesh=mesh,
            in_specs=(PartitionSpec("core"),) * (n_params + n_outs),
            out_specs=(PartitionSpec("core"),) * n_outs,
            check_rep=False,
        ),
        donate_argnums=donate,
        keep_unused=True,
    )

    def run(in_maps):
        concat_in = [
            np.concatenate([np.asarray(m[nm]) for m in in_maps], axis=0)
            for nm in in_names
        ]
        zouts = [
            np.zeros((NCORES * a.shape[0], *a.shape[1:]), a.dtype)
            for a in out_avals
        ]
        out = sharded(*concat_in, *zouts)
        # np.asarray blocks on completion; an explicit block_until_ready
        # first would cost one extra tunnel round-trip.
        return [
            {
                nm: np.asarray(out[i]).reshape(NCORES, *out_avals[i].shape)[c]
                for i, nm in enumerate(out_names)
            }
            for c in range(NCORES)
        ]

    _CACHE["runner"] = run
    return run


def kernel(embeddings, W, b, target, perm, k_pos_samples, m_neg_samples):
    k = int(k_pos_samples)
    m = min(int(m_neg_samples), k * (N - 1))
    assert k == K and m == M and embeddings.shape == (N * K, H)

    run = _get_runner()
    in_maps = _prep_inputs(embeddings, W, b, target, perm, k, m)
    res = run(in_maps)
    total = 0.0
    for c in range(NCORES):
        total += float(np.sum(res[c]["loss_part"].astype(np.float64)))
    return np.float32(total / N)
